# revision 26
# baseline (speedup 1.0000x reference)
"""Trainium2 Bass kernel for MultiHeadedAttention with clipped relative-position
bias (sparse_attention), causal mask, output projection, residual + LayerNorm.

Sharding (8 cores): data-parallel over batch (2) x tensor-parallel over head
pairs (4).  Each core projects Q/K/V for its 2 heads (dm-slice of 128), runs
full causal attention for those heads, computes its partial output projection
with its slice of Wo, and two 4-core ReduceScatters sum the partials while
scattering q-row quarters; each core then applies residual + LayerNorm to its
256-row quarter and writes it out.

Relative-position term: scores[q,k] += q . table[clip(k-q,-32,32)+32] / 8.
Under the causal mask this is a constant-per-row base (table idx 0, which
cancels in softmax) plus a 33-wide diagonal band.  The band is materialized
with a diagonal-scatter DMA into a DRAM window buffer whose persistent
background holds 0 inside the band region and -1e30 above the diagonal (which
simultaneously applies the causal mask of the diagonal 128-block).  All 16
band windows are prepared up front so the DRAM round-trip latency never sits
on the attention critical path.  The same skew trick extracts the 32-wide
probability band for the P @ R_v term, which collapses to
[1-sum(band), band] @ table[0:33] and is accumulated onto the attention
output in a deferred pass.
"""

import math
import os
import sys

import numpy as np

sys.path.insert(0, "/opt/trn_rl_repo")

import ml_dtypes  # noqa: E402

import concourse.bacc as bacc  # noqa: E402
import concourse.bass as bass  # noqa: E402
import concourse.mybir as mybir  # noqa: E402
from concourse import bass_utils  # noqa: E402
from concourse.masks import make_identity  # noqa: E402
from concourse.tile import TileContext  # noqa: E402

B, S, DM, H, DH = 2, 1024, 512, 8, 64
PCLIP = 32          # max relative position
NB = S // 128       # q-blocks per sequence
EPS = 1e-6
NEG = -1e30
BF = mybir.dt.bfloat16
F32 = mybir.dt.float32
BF_NP = ml_dtypes.bfloat16
REPLICA_GROUPS = [[0, 1, 2, 3], [4, 5, 6, 7]]


def _sincos_table():
    n = 2 * PCLIP + 1
    pos = np.arange(n)[:, None].astype(np.float32)
    div = np.exp(np.arange(0, DH, 2).astype(np.float32) * (-math.log(10000.0) / DH))
    pe = np.zeros((n, DH), dtype=np.float32)
    pe[:, 0::2] = np.sin(pos * div)
    pe[:, 1::2] = np.cos(pos * div)
    return pe  # [65, 64]


def build(loop=1):
    nc = bacc.Bacc(
        "TRN2",
        target_bir_lowering=False,
        debug=False,
        enable_asserts=False,
        num_devices=8,
    )
    xqT = nc.dram_tensor("xqT", [128, 4, S], BF, kind="ExternalInput")
    kxT = nc.dram_tensor("kxT", [128, 4, S], BF, kind="ExternalInput")
    vxT = nc.dram_tensor("vxT", [128, 4, S], BF, kind="ExternalInput")
    wqT = nc.dram_tensor("wqT", [128, 4, 128], BF, kind="ExternalInput")
    wkT = nc.dram_tensor("wkT", [128, 4, 128], BF, kind="ExternalInput")
    wvT = nc.dram_tensor("wvT", [128, 4, 128], BF, kind="ExternalInput")
    wqb = nc.dram_tensor("wqb", [1, 128], BF, kind="ExternalInput")
    wkb = nc.dram_tensor("wkb", [1, 128], BF, kind="ExternalInput")
    wvb = nc.dram_tensor("wvb", [1, 128], BF, kind="ExternalInput")
    woT = nc.dram_tensor("woT", [128, DM], BF, kind="ExternalInput")
    resid = nc.dram_tensor("resid", [256, DM], F32, kind="ExternalInput")
    lnw = nc.dram_tensor("lnw", [1, DM], F32, kind="ExternalInput")
    lnb = nc.dram_tensor("lnb", [1, DM], F32, kind="ExternalInput")
    tabT = nc.dram_tensor("tabT", [DH, 33], BF, kind="ExternalInput")
    w33 = nc.dram_tensor("w33", [64, DM], BF, kind="ExternalInput")
    bg = nc.dram_tensor("bg", [128, 160], BF, kind="ExternalInput")
    out = nc.dram_tensor("out", [256, DM], F32, kind="ExternalOutput")

    # The heavy odd q-blocks run first so their ReduceScatter overlaps the
    # lighter even-block attention iterations; h is the inner axis.
    ITERS = [(h, i) for i in (7, 5, 3, 1, 6, 4, 2, 0) for h in range(2)]

    def emit_body():
      with TileContext(nc) as tc:
        import contextlib

        with contextlib.ExitStack() as ctx:
            consts = ctx.enter_context(tc.tile_pool(name="consts", bufs=1))
            big = ctx.enter_context(tc.tile_pool(name="big", bufs=1))
            work = ctx.enter_context(tc.tile_pool(name="work", bufs=6))
            pwork = ctx.enter_context(tc.tile_pool(name="pwork", bufs=4))
            psS = ctx.enter_context(tc.tile_pool(name="psS", bufs=2, space="PSUM"))
            psA = ctx.enter_context(tc.tile_pool(name="psA", bufs=2, space="PSUM"))
            dram = ctx.enter_context(tc.tile_pool(name="dram", bufs=1, space="DRAM"))

            # ---------------- constant / input loads ----------------
            # critical path first: zeros/wq/wk/tabT -> xq -> zeroed swin scratch
            def load_w(t, tb):
                w_sb = consts.tile([128, 4, 128], BF, name=t.name + "_sb")
                nc.sync.dma_start(out=w_sb, in_=t.ap())
                w_b = consts.tile([1, 128], BF, name=t.name + "_b")
                nc.sync.dma_start(out=w_b, in_=tb.ap())
                return w_sb, w_b

            zeros_sb = consts.tile([128, 16 * 160], BF)
            nc.vector.memset(zeros_sb, 0.0)
            wq_sb, wq_b = load_w(wqT, wqb)
            wk_sb, wk_b = load_w(wkT, wkb)
            tabT_sb = consts.tile([128, 33], BF)
            nc.sync.dma_start(out=tabT_sb[0:64], in_=tabT.ap())
            nc.sync.dma_start(out=tabT_sb[64:128], in_=tabT.ap())
            xq_sb = big.tile([128, 4, S], BF)
            nc.sync.dma_start(out=xq_sb, in_=xqT.ap())

            # ---------------- DRAM scratch ----------------
            swin_all = dram.tile([128, 16 * 160], BF, tag="swin_all")
            nc.sync.dma_start(out=swin_all, in_=zeros_sb)

            kx_sb = big.tile([128, 4, S], BF)
            nc.scalar.dma_start(out=kx_sb, in_=kxT.ap())
            vx_sb = big.tile([128, 4, S], BF)
            nc.scalar.dma_start(out=vx_sb, in_=vxT.ap())
            wv_sb, wv_b = load_w(wvT, wvb)
            wo_sb = consts.tile([128, DM], BF)
            nc.sync.dma_start(out=wo_sb, in_=woT.ap())
            w33_sb = consts.tile([64, DM], BF)
            nc.sync.dma_start(out=w33_sb, in_=w33.ap())
            lnw_sb = consts.tile([128, DM], F32)
            nc.sync.dma_start(
                out=lnw_sb,
                in_=bass.AP(tensor=lnw.ap().tensor, offset=0, ap=[[0, 128], [1, DM]]),
            )
            lnb_sb = consts.tile([128, DM], F32)
            nc.sync.dma_start(
                out=lnb_sb,
                in_=bass.AP(tensor=lnb.ap().tensor, offset=0, ap=[[0, 128], [1, DM]]),
            )
            ones_sb = consts.tile([1, S], BF)
            nc.vector.memset(ones_sb, 1.0)
            ident = consts.tile([128, 128], BF)
            make_identity(nc, ident)
            bg_sb = consts.tile([128, 160], BF)
            nc.sync.dma_start(out=bg_sb, in_=bg.ap())
            pwinA = dram.tile([128, 8 * 160], BF, tag="pwinA")
            pwinB = dram.tile([128, 8 * 160], BF, tag="pwinB")
            rsA_in = dram.tile([512, DM], BF, tag="rsA_in")
            rsB_in = dram.tile([512, DM], BF, tag="rsB_in")
            rsA_out = dram.tile([128, DM], BF, tag="rsA_out")
            rsB_out = dram.tile([128, DM], BF, tag="rsB_out")

            # ---------------- Q^T / K^T projections ----------------
            qT_sb = big.tile([128, S], BF)
            kT_sb = big.tile([128, S], BF)

            def proj(dst, src, w_sb, w_b):
                for half in range(2):
                    sl = slice(512 * half, 512 * half + 512)
                    ps = psS.tile([128, 512], F32, tag="S1", name="ps_proj")
                    for kc in range(4):
                        nc.tensor.matmul(
                            ps, w_sb[:, kc], src[:, kc, sl],
                            start=(kc == 0), stop=False,
                        )
                    nc.tensor.matmul(ps, w_b, ones_sb[:, 0:512], start=False, stop=True)
                    nc.scalar.copy(out=dst[:, sl], in_=ps)

            proj(qT_sb, xq_sb, wq_sb, wq_b)

            # ---------------- band windows, all up front ----------------
            # scores band: delta[q, jj] = (q . tab[jj] - q . tab[0]) for the 33
            # diagonals; scattered diagonally over a -1e30/0 causal background.
            deltall = big.tile([128, 16, 33], BF)
            qtr_all = psS.tile([128, 2, 512], F32, tag="S2", name="qtr_all")
            for h in range(2):
                hsl = slice(64 * h, 64 * h + 64)
                for i in range(NB):
                    nc.tensor.matmul(
                        qtr_all[:, h, 33 * i : 33 * i + 33],
                        qT_sb[hsl, 128 * i : 128 * i + 128],
                        tabT_sb[hsl],
                        start=True, stop=True,
                    )
            for h in range(2):
                for i in range(NB):
                    it = ITERS.index((h, i))
                    q0 = work.tile([128, 1], F32, tag="q0", name="q0")
                    nc.vector.tensor_copy(out=q0, in_=qtr_all[:, h, 33 * i : 33 * i + 1])
                    nc.vector.tensor_scalar(
                        out=deltall[:, it],
                        in0=qtr_all[:, h, 33 * i : 33 * i + 33],
                        scalar1=q0, scalar2=None,
                        op0=mybir.AluOpType.subtract,
                    )
            nc.sync.dma_start(
                out=bass.AP(
                    tensor=swin_all.tensor, offset=swin_all.offset,
                    ap=[[2561, 128], [160, 16], [1, 33]],
                ),
                in_=deltall,
            )
            wtall = big.tile([128, 16, 160], BF)
            nc.sync.dma_start(out=wtall, in_=swin_all)
            for it in range(16):
                nc.vector.tensor_add(wtall[:, it], wtall[:, it], bg_sb)

            proj(kT_sb, kx_sb, wk_sb, wk_b)

            # ---------------- V projection (natural layout) ----------------
            v_sb = big.tile([128, NB, 128], BF)
            for rb in range(NB):
                psv = psS.tile([128, 128], F32, tag="S1", name="ps_v")
                for kc in range(4):
                    nc.tensor.matmul(
                        psv, vx_sb[:, kc, 128 * rb : 128 * rb + 128], wv_sb[:, kc],
                        start=(kc == 0), stop=False,
                    )
                nc.tensor.matmul(psv, ones_sb[:, 0:128], wv_b, start=False, stop=True)
                nc.gpsimd.tensor_copy(out=v_sb[:, rb], in_=psv)

            # ---------------- attention main loop + deferred rel-V ---------
            attnT = big.tile([128, S], BF)
            pwalls = {0: big.tile([128, 8, 160], BF, name="pwallA"),
                      1: big.tile([128, 8, 160], BF, name="pwallB")}
            nc.vector.memset(pwalls[1][:, 6:8], 0.0)  # i==0 zero left margins

            def main_iter(itl, it, h, i, pwall, pe_transpose=False):
                hsl = slice(64 * h, 64 * h + 64)
                Lk = 128 * (i + 1)
                q_lhsT = qT_sb[hsl, 128 * i : 128 * i + 128]
                Sp = psS.tile(
                    [128, 512 if i < 4 else S], F32,
                    tag="S1" if i < 4 else "S2", name="Sp",
                )
                for c0 in range(0, Lk, 512):
                    w = min(512, Lk - c0)
                    nc.tensor.matmul(
                        Sp[:, c0 : c0 + w], q_lhsT, kT_sb[hsl, c0 : c0 + w],
                        start=True, stop=True,
                    )
                if i == 0:
                    nc.vector.tensor_add(
                        Sp[:, 0:128], Sp[:, 0:128], wtall[:, it, 32:160]
                    )
                else:
                    c0 = 128 * i - 32
                    nc.vector.tensor_add(
                        Sp[:, c0 : c0 + 160], Sp[:, c0 : c0 + 160], wtall[:, it]
                    )
                P = pwork.tile([128, S], BF, tag="P", name="P")
                Zs = work.tile([128, 1], F32, tag="Z", name="Zs")
                nc.scalar.activation(
                    out=P[:, 0:Lk], in_=Sp[:, 0:Lk],
                    func=mybir.ActivationFunctionType.Exp, accum_out=Zs,
                )
                rz = work.tile([128, 1], F32, tag="rz", name="rz")
                nc.vector.reciprocal(out=rz, in_=Zs)
                nc.vector.tensor_scalar_mul(P[:, 0:Lk], P[:, 0:Lk], rz)
                if i == 0:
                    nc.vector.tensor_copy(out=pwall[:, itl, 32:160], in_=P[:, 0:128])
                else:
                    nc.vector.tensor_copy(
                        out=pwall[:, itl], in_=P[:, 128 * i - 32 : 128 * i + 128]
                    )
                ovT = psA.tile([128, 128], F32, tag="ps512", name="ovT")
                for kb in range(i + 1):
                    ptT = work.tile([128, 128], BF, tag="ptT", name="ptT")
                    if pe_transpose:
                        ptT_ps = psS.tile(
                            [128, 128], BF,
                            tag="S2" if i < 4 else "S1", name="ptT_ps",
                        )
                        nc.tensor.transpose(
                            ptT_ps, P[:, 128 * kb : 128 * kb + 128], ident
                        )
                        nc.scalar.copy(out=ptT, in_=ptT_ps)
                    else:
                        nc.sync.dma_start_transpose(
                            ptT, P[:, 128 * kb : 128 * kb + 128]
                        )
                    nc.tensor.matmul(
                        ovT[hsl], v_sb[:, kb, hsl], ptT,
                        start=(kb == 0), stop=(kb == i),
                    )
                nc.vector.tensor_copy(
                    out=attnT[hsl, 128 * i : 128 * i + 128], in_=ovT[hsl]
                )

            def rel_tail(group_iters, pwall, pwin):
                # single batched skew-gather of all 8 windows' probability
                # bands; out_rel = T33[0] + Pband @ (T33[1:] - T33[0]) (row
                # sums are 1), with both terms folded downstream: the constant
                # into `resid` on the host, the band product into the partial
                # output projection via the host-precomputed w33.
                nc.sync.dma_start(out=pwin, in_=pwall)
                pcall = work.tile([128, 8, 32], BF, tag="pcall", name="pcall")
                nc.sync.dma_start(
                    out=pcall,
                    in_=bass.AP(
                        tensor=pwin.tensor, offset=pwin.offset + 1,
                        ap=[[1281, 128], [160, 8], [1, 32]],
                    ),
                )
                pcTs_of = {}
                for bt in range(4):  # transpose both heads' bands of block i
                    pcT_ps = psS.tile([64, 128], BF, tag="S1", name="pcT_ps")
                    nc.tensor.transpose(pcT_ps, pcall[:, 2 * bt : 2 * bt + 2], ident)
                    pcTs = work.tile([64, 128], BF, tag="pcT", name="pcTs")
                    nc.scalar.copy(out=pcTs, in_=pcT_ps)
                    pcTs_of[group_iters[2 * bt][1]] = pcTs
                return pcTs_of

            def partial(rb, rs_buf, pcTs):
                pp = psA.tile([128, DM], F32, tag="ps512", name="pp")
                nc.tensor.matmul(
                    pp, attnT[:, 128 * rb : 128 * rb + 128], wo_sb,
                    start=True, stop=False,
                )
                nc.tensor.matmul(
                    pp, pcTs[0:32], w33_sb[0:32], start=False, stop=False
                )
                nc.tensor.matmul(
                    pp, pcTs[32:64], w33_sb[32:64], start=False, stop=True
                )
                pps = work.tile([128, DM], BF, tag="pps", name="pps")
                nc.scalar.copy(out=pps, in_=pp)
                g = rb // 2
                nc.sync.dma_start(out=rs_buf[128 * g : 128 * g + 128, :], in_=pps)

            for itl, (h, i) in enumerate(ITERS[0:8]):
                main_iter(itl, itl, h, i, pwalls[0])
            pcTs_odd = rel_tail(ITERS[0:8], pwalls[0], pwinA)
            for rb in (1, 3, 5, 7):
                partial(rb, rsA_in, pcTs_odd[rb])
            nc.gpsimd.collective_compute(
                "ReduceScatter", mybir.AluOpType.add,
                ins=[rsA_in.opt()], outs=[rsA_out.opt()],
                replica_groups=REPLICA_GROUPS,
            )
            for itl, (h, i) in enumerate(ITERS[8:16]):
                main_iter(itl, itl + 8, h, i, pwalls[1], pe_transpose=True)
            pcTs_even = rel_tail(ITERS[8:16], pwalls[1], pwinB)
            for rb in (0, 2, 4, 6):
                partial(rb, rsB_in, pcTs_even[rb])
            nc.gpsimd.collective_compute(
                "ReduceScatter", mybir.AluOpType.add,
                ins=[rsB_in.opt()], outs=[rsB_out.opt()],
                replica_groups=REPLICA_GROUPS,
            )

            # ---------------- residual + LayerNorm on this core's quarter ---
            for rb, rs_out_buf in ((1, rsA_out), (0, rsB_out)):
                rsl = slice(128 * rb, 128 * rb + 128)
                rsb = work.tile([128, DM], BF, tag="rsb", name="rsb")
                nc.sync.dma_start(out=rsb, in_=rs_out_buf)
                rst = work.tile([128, DM], F32, tag="rst", name="rst")
                nc.sync.dma_start(out=rst, in_=resid.ap()[rsl])
                xsb = work.tile([128, DM], F32, tag="xsb", name="xsb")
                nc.vector.tensor_add(xsb, rst, rsb)
                stats = work.tile([128, 6], F32, tag="stats", name="stats")
                nc.vector.bn_stats(out=stats, in_=xsb)
                mv = work.tile([128, 2], F32, tag="mv", name="mv")
                nc.vector.bn_aggr(out=mv, in_=stats)
                stde = work.tile([128, 1], F32, tag="stde", name="stde")
                nc.scalar.activation(
                    out=stde, in_=mv[:, 1:2],
                    func=mybir.ActivationFunctionType.Sqrt,
                    scale=float(DM) / float(DM - 1),
                )
                nc.vector.tensor_scalar_add(stde, stde, EPS)
                rstd = work.tile([128, 1], F32, tag="rstd", name="rstd")
                nc.vector.reciprocal(out=rstd, in_=stde)
                ysb = work.tile([128, DM], F32, tag="ysb", name="ysb")
                nc.vector.tensor_scalar(
                    out=ysb, in0=xsb, scalar1=mv[:, 0:1], scalar2=rstd,
                    op0=mybir.AluOpType.subtract, op1=mybir.AluOpType.mult,
                )
                nc.vector.tensor_mul(ysb, ysb, lnw_sb)
                nc.vector.tensor_add(ysb, ysb, lnb_sb)
                nc.sync.dma_start(out=out.ap()[rsl], in_=ysb)

    for _ in range(loop):
        emit_body()
    nc.compile()
    return nc


def make_in_maps(query, key, value, Wq, bq, Wk, bk, Wv, bv, Wo, bo, ln_w, ln_b):
    table = _sincos_table()
    tabT = np.ascontiguousarray(table.T[:, :33]).astype(BF_NP)  # [64, 33]
    t33d = table[1:33] - table[0:1]                              # [32, 64]
    rel_const = np.tile(table[0], H) @ Wo.T                      # [512]
    bgp = np.zeros((128, 160), dtype=np.float32)
    cidx = np.arange(160)[None, :]
    pidx = np.arange(128)[:, None]
    bgp[cidx > pidx + 32] = NEG
    bgp = bgp.astype(BF_NP)

    def pmajor(a):  # [512, n] -> [128, 4, n] partition-major
        n = a.shape[1]
        return np.ascontiguousarray(
            a.reshape(4, 128, n).transpose(1, 0, 2)
        ).astype(BF_NP)

    in_maps = []
    for c in range(8):
        b, g = divmod(c, 4)
        sl = slice(128 * g, 128 * g + 128)
        resid = (
            query[b, 256 * g : 256 * g + 256, :] + bo[None, :] + rel_const[None, :]
        ).astype(np.float32)
        woT_sl = Wo.T[sl, :]
        w33_c = np.concatenate([t33d @ woT_sl[0:64], t33d @ woT_sl[64:128]], axis=0)
        in_maps.append(
            {
                "xqT": pmajor(np.ascontiguousarray(query[b].T)),
                "kxT": pmajor(np.ascontiguousarray(key[b].T)),
                "vxT": pmajor(np.ascontiguousarray(value[b].T)),
                "wqT": pmajor(np.ascontiguousarray(Wq.T[:, sl] / 8.0)),
                "wkT": pmajor(np.ascontiguousarray(Wk.T[:, sl])),
                "wvT": pmajor(np.ascontiguousarray(Wv.T[:, sl])),
                "wqb": (bq[sl] / 8.0)[None, :].astype(BF_NP),
                "wkb": bk[sl][None, :].astype(BF_NP),
                "wvb": bv[sl][None, :].astype(BF_NP),
                "woT": np.ascontiguousarray(Wo.T[sl, :]).astype(BF_NP),
                "resid": resid,
                "lnw": np.asarray(ln_w, np.float32).reshape(1, DM),
                "lnb": np.asarray(ln_b, np.float32).reshape(1, DM),
                "tabT": tabT,
                "w33": np.ascontiguousarray(w33_c).astype(BF_NP),
                "bg": bgp,
            }
        )
    return in_maps


def assemble(results):
    full = np.zeros((B, S, DM), dtype=np.float32)
    for c in range(8):
        b, g = divmod(c, 4)
        full[b, 256 * g : 256 * g + 256, :] = results[c]["out"]
    return full


_CACHE = {}


def kernel(**inputs):
    inputs = {k: np.asarray(v) for k, v in inputs.items()}
    if "nc" not in _CACHE:
        _CACHE["nc"] = build()
    nc = _CACHE["nc"]
    in_maps = make_in_maps(**inputs)
    res = bass_utils.run_bass_kernel_spmd(nc, in_maps, core_ids=list(range(8)))
    return assemble(res.results)


if __name__ == "__main__":
    rng = np.random.default_rng(0)
    dummy = {
        "query": rng.normal(size=(B, S, DM)).astype(np.float32),
        "key": rng.normal(size=(B, S, DM)).astype(np.float32),
        "value": rng.normal(size=(B, S, DM)).astype(np.float32),
        "Wq": rng.normal(size=(DM, DM)).astype(np.float32) * 0.02,
        "bq": np.zeros(DM, np.float32),
        "Wk": rng.normal(size=(DM, DM)).astype(np.float32) * 0.02,
        "bk": np.zeros(DM, np.float32),
        "Wv": rng.normal(size=(DM, DM)).astype(np.float32) * 0.02,
        "bv": np.zeros(DM, np.float32),
        "Wo": rng.normal(size=(DM, DM)).astype(np.float32) * 0.02,
        "bo": np.zeros(DM, np.float32),
        "ln_w": np.ones(DM, np.float32),
        "ln_b": np.zeros(DM, np.float32),
    }
    out = kernel(**dummy)
    print("kernel ran, out shape", out.shape, "mean", float(np.abs(out).mean()))


# revision 27
# speedup vs baseline: 1.0133x; 1.0133x over previous
"""Trainium2 Bass kernel for MultiHeadedAttention with clipped relative-position
bias (sparse_attention), causal mask, output projection, residual + LayerNorm.

Sharding (8 cores): data-parallel over batch (2) x tensor-parallel over head
pairs (4).  Each core projects Q/K/V for its 2 heads (dm-slice of 128), runs
full causal attention for those heads, computes its partial output projection
with its slice of Wo, and two 4-core ReduceScatters sum the partials while
scattering q-row quarters; each core then applies residual + LayerNorm to its
256-row quarter and writes it out.

Relative-position term: scores[q,k] += q . table[clip(k-q,-32,32)+32] / 8.
Under the causal mask this is a constant-per-row base (table idx 0, which
cancels in softmax) plus a 33-wide diagonal band.  The band is materialized
with a diagonal-scatter DMA into a DRAM window buffer whose persistent
background holds 0 inside the band region and -1e30 above the diagonal (which
simultaneously applies the causal mask of the diagonal 128-block).  All 16
band windows are prepared up front so the DRAM round-trip latency never sits
on the attention critical path.  The same skew trick extracts the 32-wide
probability band for the P @ R_v term, which collapses to
[1-sum(band), band] @ table[0:33] and is accumulated onto the attention
output in a deferred pass.
"""

import math
import os
import sys

import numpy as np

sys.path.insert(0, "/opt/trn_rl_repo")

import ml_dtypes  # noqa: E402

import concourse.bacc as bacc  # noqa: E402
import concourse.bass as bass  # noqa: E402
import concourse.mybir as mybir  # noqa: E402
from concourse import bass_utils  # noqa: E402
from concourse.masks import make_identity  # noqa: E402
from concourse.tile import TileContext  # noqa: E402

B, S, DM, H, DH = 2, 1024, 512, 8, 64
PCLIP = 32          # max relative position
NB = S // 128       # q-blocks per sequence
EPS = 1e-6
NEG = -1e30
BF = mybir.dt.bfloat16
F32 = mybir.dt.float32
BF_NP = ml_dtypes.bfloat16
REPLICA_GROUPS = [[0, 1, 2, 3], [4, 5, 6, 7]]


def _sincos_table():
    n = 2 * PCLIP + 1
    pos = np.arange(n)[:, None].astype(np.float32)
    div = np.exp(np.arange(0, DH, 2).astype(np.float32) * (-math.log(10000.0) / DH))
    pe = np.zeros((n, DH), dtype=np.float32)
    pe[:, 0::2] = np.sin(pos * div)
    pe[:, 1::2] = np.cos(pos * div)
    return pe  # [65, 64]


def build(loop=1):
    nc = bacc.Bacc(
        "TRN2",
        target_bir_lowering=False,
        debug=False,
        enable_asserts=False,
        num_devices=8,
    )
    xqT = nc.dram_tensor("xqT", [128, 4, S], BF, kind="ExternalInput")
    kxT = nc.dram_tensor("kxT", [128, 4, S], BF, kind="ExternalInput")
    vxT = nc.dram_tensor("vxT", [128, 4, S], BF, kind="ExternalInput")
    wqT = nc.dram_tensor("wqT", [128, 4, 128], BF, kind="ExternalInput")
    wkT = nc.dram_tensor("wkT", [128, 4, 128], BF, kind="ExternalInput")
    wvT = nc.dram_tensor("wvT", [128, 4, 128], BF, kind="ExternalInput")
    wqb = nc.dram_tensor("wqb", [1, 128], BF, kind="ExternalInput")
    wkb = nc.dram_tensor("wkb", [1, 128], BF, kind="ExternalInput")
    wvb = nc.dram_tensor("wvb", [1, 128], BF, kind="ExternalInput")
    woT = nc.dram_tensor("woT", [128, DM], BF, kind="ExternalInput")
    resid = nc.dram_tensor("resid", [256, DM], F32, kind="ExternalInput")
    lnw = nc.dram_tensor("lnw", [1, DM], F32, kind="ExternalInput")
    lnb = nc.dram_tensor("lnb", [1, DM], F32, kind="ExternalInput")
    tabT = nc.dram_tensor("tabT", [DH, 33], BF, kind="ExternalInput")
    w33 = nc.dram_tensor("w33", [64, DM], BF, kind="ExternalInput")
    bg = nc.dram_tensor("bg", [128, 160], BF, kind="ExternalInput")
    out = nc.dram_tensor("out", [256, DM], F32, kind="ExternalOutput")

    # The heavy odd q-blocks run first so their ReduceScatter overlaps the
    # lighter even-block attention iterations; h is the inner axis.
    ITERS = [(h, i) for i in (1, 3, 5, 7, 0, 2, 4, 6) for h in range(2)]

    def emit_body():
      with TileContext(nc) as tc:
        import contextlib

        with contextlib.ExitStack() as ctx:
            consts = ctx.enter_context(tc.tile_pool(name="consts", bufs=1))
            big = ctx.enter_context(tc.tile_pool(name="big", bufs=1))
            work = ctx.enter_context(tc.tile_pool(name="work", bufs=6))
            pwork = ctx.enter_context(tc.tile_pool(name="pwork", bufs=4))
            psS = ctx.enter_context(tc.tile_pool(name="psS", bufs=2, space="PSUM"))
            psA = ctx.enter_context(tc.tile_pool(name="psA", bufs=2, space="PSUM"))
            dram = ctx.enter_context(tc.tile_pool(name="dram", bufs=1, space="DRAM"))

            # ---------------- constant / input loads ----------------
            # critical path first: zeros/wq/wk/tabT -> xq -> zeroed swin scratch
            def load_w(t, tb):
                w_sb = consts.tile([128, 4, 128], BF, name=t.name + "_sb")
                nc.sync.dma_start(out=w_sb, in_=t.ap())
                w_b = consts.tile([1, 128], BF, name=t.name + "_b")
                nc.sync.dma_start(out=w_b, in_=tb.ap())
                return w_sb, w_b

            zeros_sb = consts.tile([128, 16 * 160], BF)
            nc.vector.memset(zeros_sb, 0.0)
            wq_sb, wq_b = load_w(wqT, wqb)
            wk_sb, wk_b = load_w(wkT, wkb)
            tabT_sb = consts.tile([128, 33], BF)
            nc.sync.dma_start(out=tabT_sb[0:64], in_=tabT.ap())
            nc.sync.dma_start(out=tabT_sb[64:128], in_=tabT.ap())
            xq_sb = big.tile([128, 4, S], BF)
            nc.sync.dma_start(out=xq_sb, in_=xqT.ap())

            # ---------------- DRAM scratch ----------------
            swin_all = dram.tile([128, 16 * 160], BF, tag="swin_all")
            nc.sync.dma_start(out=swin_all, in_=zeros_sb)

            kx_sb = big.tile([128, 4, S], BF)
            nc.scalar.dma_start(out=kx_sb, in_=kxT.ap())
            vx_sb = big.tile([128, 4, S], BF)
            nc.scalar.dma_start(out=vx_sb, in_=vxT.ap())
            wv_sb, wv_b = load_w(wvT, wvb)
            wo_sb = consts.tile([128, DM], BF)
            nc.sync.dma_start(out=wo_sb, in_=woT.ap())
            w33_sb = consts.tile([64, DM], BF)
            nc.sync.dma_start(out=w33_sb, in_=w33.ap())
            lnw_sb = consts.tile([128, DM], F32)
            nc.sync.dma_start(
                out=lnw_sb,
                in_=bass.AP(tensor=lnw.ap().tensor, offset=0, ap=[[0, 128], [1, DM]]),
            )
            lnb_sb = consts.tile([128, DM], F32)
            nc.sync.dma_start(
                out=lnb_sb,
                in_=bass.AP(tensor=lnb.ap().tensor, offset=0, ap=[[0, 128], [1, DM]]),
            )
            ones_sb = consts.tile([1, S], BF)
            nc.vector.memset(ones_sb, 1.0)
            ident = consts.tile([128, 128], BF)
            make_identity(nc, ident)
            bg_sb = consts.tile([128, 160], BF)
            nc.sync.dma_start(out=bg_sb, in_=bg.ap())
            pwinA = dram.tile([128, 8 * 160], BF, tag="pwinA")
            pwinB = dram.tile([128, 8 * 160], BF, tag="pwinB")
            rsA_in = dram.tile([512, DM], BF, tag="rsA_in")
            rsB_in = dram.tile([512, DM], BF, tag="rsB_in")
            rsA_out = dram.tile([128, DM], BF, tag="rsA_out")
            rsB_out = dram.tile([128, DM], BF, tag="rsB_out")

            # ---------------- Q^T / K^T projections ----------------
            qT_sb = big.tile([128, S], BF)
            kT_sb = big.tile([128, S], BF)

            def proj(dst, src, w_sb, w_b):
                for half in range(2):
                    sl = slice(512 * half, 512 * half + 512)
                    ps = psS.tile([128, 512], F32, tag="S1", name="ps_proj")
                    for kc in range(4):
                        nc.tensor.matmul(
                            ps, w_sb[:, kc], src[:, kc, sl],
                            start=(kc == 0), stop=False,
                        )
                    nc.tensor.matmul(ps, w_b, ones_sb[:, 0:512], start=False, stop=True)
                    nc.scalar.copy(out=dst[:, sl], in_=ps)

            proj(qT_sb, xq_sb, wq_sb, wq_b)

            # ---------------- band windows, all up front ----------------
            # scores band: delta[q, jj] = (q . tab[jj] - q . tab[0]) for the 33
            # diagonals; scattered diagonally over a -1e30/0 causal background.
            deltall = big.tile([128, 16, 33], BF)
            qtr_all = psS.tile([128, 2, 512], F32, tag="S2", name="qtr_all")
            for h in range(2):
                hsl = slice(64 * h, 64 * h + 64)
                for i in range(NB):
                    nc.tensor.matmul(
                        qtr_all[:, h, 33 * i : 33 * i + 33],
                        qT_sb[hsl, 128 * i : 128 * i + 128],
                        tabT_sb[hsl],
                        start=True, stop=True,
                    )
            for h in range(2):
                for i in range(NB):
                    it = ITERS.index((h, i))
                    q0 = work.tile([128, 1], F32, tag="q0", name="q0")
                    nc.vector.tensor_copy(out=q0, in_=qtr_all[:, h, 33 * i : 33 * i + 1])
                    nc.vector.tensor_scalar(
                        out=deltall[:, it],
                        in0=qtr_all[:, h, 33 * i : 33 * i + 33],
                        scalar1=q0, scalar2=None,
                        op0=mybir.AluOpType.subtract,
                    )
            nc.sync.dma_start(
                out=bass.AP(
                    tensor=swin_all.tensor, offset=swin_all.offset,
                    ap=[[2561, 128], [160, 16], [1, 33]],
                ),
                in_=deltall,
            )
            wtall = big.tile([128, 16, 160], BF)
            nc.sync.dma_start(out=wtall, in_=swin_all)
            for it in range(16):
                nc.vector.tensor_add(wtall[:, it], wtall[:, it], bg_sb)

            proj(kT_sb, kx_sb, wk_sb, wk_b)

            # ---------------- V projection (natural layout) ----------------
            v_sb = big.tile([128, NB, 128], BF)
            for rb in range(NB):
                psv = psS.tile([128, 128], F32, tag="S1", name="ps_v")
                for kc in range(4):
                    nc.tensor.matmul(
                        psv, vx_sb[:, kc, 128 * rb : 128 * rb + 128], wv_sb[:, kc],
                        start=(kc == 0), stop=False,
                    )
                nc.tensor.matmul(psv, ones_sb[:, 0:128], wv_b, start=False, stop=True)
                nc.gpsimd.tensor_copy(out=v_sb[:, rb], in_=psv)

            # ---------------- attention main loop + deferred rel-V ---------
            attnT = big.tile([128, S], BF)
            pwalls = {0: big.tile([128, 8, 160], BF, name="pwallA"),
                      1: big.tile([128, 8, 160], BF, name="pwallB")}
            nc.vector.memset(pwalls[1][:, 0:2], 0.0)  # i==0 zero left margins

            def main_iter(itl, it, h, i, pwall, pe_transpose=False):
                hsl = slice(64 * h, 64 * h + 64)
                Lk = 128 * (i + 1)
                q_lhsT = qT_sb[hsl, 128 * i : 128 * i + 128]
                Sp = psS.tile(
                    [128, 512 if i < 4 else S], F32,
                    tag="S1" if i < 4 else "S2", name="Sp",
                )
                for c0 in range(0, Lk, 512):
                    w = min(512, Lk - c0)
                    nc.tensor.matmul(
                        Sp[:, c0 : c0 + w], q_lhsT, kT_sb[hsl, c0 : c0 + w],
                        start=True, stop=True,
                    )
                if i == 0:
                    nc.vector.tensor_add(
                        Sp[:, 0:128], Sp[:, 0:128], wtall[:, it, 32:160]
                    )
                else:
                    c0 = 128 * i - 32
                    nc.vector.tensor_add(
                        Sp[:, c0 : c0 + 160], Sp[:, c0 : c0 + 160], wtall[:, it]
                    )
                P = pwork.tile([128, S], BF, tag="P", name="P")
                Zs = work.tile([128, 1], F32, tag="Z", name="Zs")
                nc.scalar.activation(
                    out=P[:, 0:Lk], in_=Sp[:, 0:Lk],
                    func=mybir.ActivationFunctionType.Exp, accum_out=Zs,
                )
                rz = work.tile([128, 1], F32, tag="rz", name="rz")
                nc.vector.reciprocal(out=rz, in_=Zs)
                nc.vector.tensor_scalar_mul(P[:, 0:Lk], P[:, 0:Lk], rz)
                if i == 0:
                    nc.vector.tensor_copy(out=pwall[:, itl, 32:160], in_=P[:, 0:128])
                else:
                    nc.vector.tensor_copy(
                        out=pwall[:, itl], in_=P[:, 128 * i - 32 : 128 * i + 128]
                    )
                ovT = psA.tile([128, 128], F32, tag="ps512", name="ovT")
                for kb in range(i + 1):
                    ptT = work.tile([128, 128], BF, tag="ptT", name="ptT")
                    if pe_transpose:
                        ptT_ps = psS.tile(
                            [128, 128], BF,
                            tag="S2" if i < 4 else "S1", name="ptT_ps",
                        )
                        nc.tensor.transpose(
                            ptT_ps, P[:, 128 * kb : 128 * kb + 128], ident
                        )
                        nc.scalar.copy(out=ptT, in_=ptT_ps)
                    else:
                        nc.sync.dma_start_transpose(
                            ptT, P[:, 128 * kb : 128 * kb + 128]
                        )
                    nc.tensor.matmul(
                        ovT[hsl], v_sb[:, kb, hsl], ptT,
                        start=(kb == 0), stop=(kb == i),
                    )
                nc.vector.tensor_copy(
                    out=attnT[hsl, 128 * i : 128 * i + 128], in_=ovT[hsl]
                )

            def rel_tail(group_iters, pwall, pwin):
                # single batched skew-gather of all 8 windows' probability
                # bands; out_rel = T33[0] + Pband @ (T33[1:] - T33[0]) (row
                # sums are 1), with both terms folded downstream: the constant
                # into `resid` on the host, the band product into the partial
                # output projection via the host-precomputed w33.
                nc.sync.dma_start(out=pwin, in_=pwall)
                pcall = work.tile([128, 8, 32], BF, tag="pcall", name="pcall")
                nc.sync.dma_start(
                    out=pcall,
                    in_=bass.AP(
                        tensor=pwin.tensor, offset=pwin.offset + 1,
                        ap=[[1281, 128], [160, 8], [1, 32]],
                    ),
                )
                pcTs_of = {}
                for bt in range(4):  # transpose both heads' bands of block i
                    pcT_ps = psS.tile([64, 128], BF, tag="S1", name="pcT_ps")
                    nc.tensor.transpose(pcT_ps, pcall[:, 2 * bt : 2 * bt + 2], ident)
                    pcTs = work.tile([64, 128], BF, tag="pcT", name="pcTs")
                    nc.scalar.copy(out=pcTs, in_=pcT_ps)
                    pcTs_of[group_iters[2 * bt][1]] = pcTs
                return pcTs_of

            def partial(rb, rs_buf, pcTs):
                pp = psA.tile([128, DM], F32, tag="ps512", name="pp")
                nc.tensor.matmul(
                    pp, attnT[:, 128 * rb : 128 * rb + 128], wo_sb,
                    start=True, stop=False,
                )
                nc.tensor.matmul(
                    pp, pcTs[0:32], w33_sb[0:32], start=False, stop=False
                )
                nc.tensor.matmul(
                    pp, pcTs[32:64], w33_sb[32:64], start=False, stop=True
                )
                pps = work.tile([128, DM], BF, tag="pps", name="pps")
                nc.scalar.copy(out=pps, in_=pp)
                g = rb // 2
                nc.sync.dma_start(out=rs_buf[128 * g : 128 * g + 128, :], in_=pps)

            for itl, (h, i) in enumerate(ITERS[0:8]):
                main_iter(itl, itl, h, i, pwalls[0])
            pcTs_odd = rel_tail(ITERS[0:8], pwalls[0], pwinA)
            for rb in (1, 3, 5, 7):
                partial(rb, rsA_in, pcTs_odd[rb])
            nc.gpsimd.collective_compute(
                "ReduceScatter", mybir.AluOpType.add,
                ins=[rsA_in.opt()], outs=[rsA_out.opt()],
                replica_groups=REPLICA_GROUPS,
            )
            for itl, (h, i) in enumerate(ITERS[8:16]):
                main_iter(itl, itl + 8, h, i, pwalls[1], pe_transpose=True)
            pcTs_even = rel_tail(ITERS[8:16], pwalls[1], pwinB)
            for rb in (0, 2, 4, 6):
                partial(rb, rsB_in, pcTs_even[rb])
            nc.gpsimd.collective_compute(
                "ReduceScatter", mybir.AluOpType.add,
                ins=[rsB_in.opt()], outs=[rsB_out.opt()],
                replica_groups=REPLICA_GROUPS,
            )

            # ---------------- residual + LayerNorm on this core's quarter ---
            for rb, rs_out_buf in ((1, rsA_out), (0, rsB_out)):
                rsl = slice(128 * rb, 128 * rb + 128)
                rsb = work.tile([128, DM], BF, tag="rsb", name="rsb")
                nc.sync.dma_start(out=rsb, in_=rs_out_buf)
                rst = work.tile([128, DM], F32, tag="rst", name="rst")
                nc.sync.dma_start(out=rst, in_=resid.ap()[rsl])
                xsb = work.tile([128, DM], F32, tag="xsb", name="xsb")
                nc.vector.tensor_add(xsb, rst, rsb)
                stats = work.tile([128, 6], F32, tag="stats", name="stats")
                nc.vector.bn_stats(out=stats, in_=xsb)
                mv = work.tile([128, 2], F32, tag="mv", name="mv")
                nc.vector.bn_aggr(out=mv, in_=stats)
                stde = work.tile([128, 1], F32, tag="stde", name="stde")
                nc.scalar.activation(
                    out=stde, in_=mv[:, 1:2],
                    func=mybir.ActivationFunctionType.Sqrt,
                    scale=float(DM) / float(DM - 1),
                )
                nc.vector.tensor_scalar_add(stde, stde, EPS)
                rstd = work.tile([128, 1], F32, tag="rstd", name="rstd")
                nc.vector.reciprocal(out=rstd, in_=stde)
                ysb = work.tile([128, DM], F32, tag="ysb", name="ysb")
                nc.vector.tensor_scalar(
                    out=ysb, in0=xsb, scalar1=mv[:, 0:1], scalar2=rstd,
                    op0=mybir.AluOpType.subtract, op1=mybir.AluOpType.mult,
                )
                nc.vector.tensor_mul(ysb, ysb, lnw_sb)
                nc.vector.tensor_add(ysb, ysb, lnb_sb)
                nc.sync.dma_start(out=out.ap()[rsl], in_=ysb)

    for _ in range(loop):
        emit_body()
    nc.compile()
    return nc


def make_in_maps(query, key, value, Wq, bq, Wk, bk, Wv, bv, Wo, bo, ln_w, ln_b):
    table = _sincos_table()
    tabT = np.ascontiguousarray(table.T[:, :33]).astype(BF_NP)  # [64, 33]
    t33d = table[1:33] - table[0:1]                              # [32, 64]
    rel_const = np.tile(table[0], H) @ Wo.T                      # [512]
    bgp = np.zeros((128, 160), dtype=np.float32)
    cidx = np.arange(160)[None, :]
    pidx = np.arange(128)[:, None]
    bgp[cidx > pidx + 32] = NEG
    bgp = bgp.astype(BF_NP)

    def pmajor(a):  # [512, n] -> [128, 4, n] partition-major
        n = a.shape[1]
        return np.ascontiguousarray(
            a.reshape(4, 128, n).transpose(1, 0, 2)
        ).astype(BF_NP)

    in_maps = []
    for c in range(8):
        b, g = divmod(c, 4)
        sl = slice(128 * g, 128 * g + 128)
        resid = (
            query[b, 256 * g : 256 * g + 256, :] + bo[None, :] + rel_const[None, :]
        ).astype(np.float32)
        woT_sl = Wo.T[sl, :]
        w33_c = np.concatenate([t33d @ woT_sl[0:64], t33d @ woT_sl[64:128]], axis=0)
        in_maps.append(
            {
                "xqT": pmajor(np.ascontiguousarray(query[b].T)),
                "kxT": pmajor(np.ascontiguousarray(key[b].T)),
                "vxT": pmajor(np.ascontiguousarray(value[b].T)),
                "wqT": pmajor(np.ascontiguousarray(Wq.T[:, sl] / 8.0)),
                "wkT": pmajor(np.ascontiguousarray(Wk.T[:, sl])),
                "wvT": pmajor(np.ascontiguousarray(Wv.T[:, sl])),
                "wqb": (bq[sl] / 8.0)[None, :].astype(BF_NP),
                "wkb": bk[sl][None, :].astype(BF_NP),
                "wvb": bv[sl][None, :].astype(BF_NP),
                "woT": np.ascontiguousarray(Wo.T[sl, :]).astype(BF_NP),
                "resid": resid,
                "lnw": np.asarray(ln_w, np.float32).reshape(1, DM),
                "lnb": np.asarray(ln_b, np.float32).reshape(1, DM),
                "tabT": tabT,
                "w33": np.ascontiguousarray(w33_c).astype(BF_NP),
                "bg": bgp,
            }
        )
    return in_maps


def assemble(results):
    full = np.zeros((B, S, DM), dtype=np.float32)
    for c in range(8):
        b, g = divmod(c, 4)
        full[b, 256 * g : 256 * g + 256, :] = results[c]["out"]
    return full


_CACHE = {}


def kernel(**inputs):
    inputs = {k: np.asarray(v) for k, v in inputs.items()}
    if "nc" not in _CACHE:
        _CACHE["nc"] = build()
    nc = _CACHE["nc"]
    in_maps = make_in_maps(**inputs)
    res = bass_utils.run_bass_kernel_spmd(nc, in_maps, core_ids=list(range(8)))
    return assemble(res.results)


if __name__ == "__main__":
    rng = np.random.default_rng(0)
    dummy = {
        "query": rng.normal(size=(B, S, DM)).astype(np.float32),
        "key": rng.normal(size=(B, S, DM)).astype(np.float32),
        "value": rng.normal(size=(B, S, DM)).astype(np.float32),
        "Wq": rng.normal(size=(DM, DM)).astype(np.float32) * 0.02,
        "bq": np.zeros(DM, np.float32),
        "Wk": rng.normal(size=(DM, DM)).astype(np.float32) * 0.02,
        "bk": np.zeros(DM, np.float32),
        "Wv": rng.normal(size=(DM, DM)).astype(np.float32) * 0.02,
        "bv": np.zeros(DM, np.float32),
        "Wo": rng.normal(size=(DM, DM)).astype(np.float32) * 0.02,
        "bo": np.zeros(DM, np.float32),
        "ln_w": np.ones(DM, np.float32),
        "ln_b": np.zeros(DM, np.float32),
    }
    out = kernel(**dummy)
    print("kernel ran, out shape", out.shape, "mean", float(np.abs(out).mean()))


# revision 28
# speedup vs baseline: 1.0373x; 1.0236x over previous
"""Trainium2 Bass kernel for MultiHeadedAttention with clipped relative-position
bias (sparse_attention), causal mask, output projection, residual + LayerNorm.

Sharding (8 cores): data-parallel over batch (2) x tensor-parallel over head
pairs (4).  Each core projects Q/K/V for its 2 heads (dm-slice of 128), runs
full causal attention for those heads, computes its partial output projection
with its slice of Wo, and two 4-core ReduceScatters sum the partials while
scattering q-row quarters; each core then applies residual + LayerNorm to its
256-row quarter and writes it out.

Relative-position term: scores[q,k] += q . table[clip(k-q,-32,32)+32] / 8.
Under the causal mask this is a constant-per-row base (table idx 0, which
cancels in softmax) plus a 33-wide diagonal band.  The band is materialized
with a diagonal-scatter DMA into a DRAM window buffer whose persistent
background holds 0 inside the band region and -1e30 above the diagonal (which
simultaneously applies the causal mask of the diagonal 128-block).  All 16
band windows are prepared up front so the DRAM round-trip latency never sits
on the attention critical path.  The same skew trick extracts the 32-wide
probability band for the P @ R_v term, which collapses to
[1-sum(band), band] @ table[0:33] and is accumulated onto the attention
output in a deferred pass.
"""

import math
import os
import sys

import numpy as np

sys.path.insert(0, "/opt/trn_rl_repo")

import ml_dtypes  # noqa: E402

import concourse.bacc as bacc  # noqa: E402
import concourse.bass as bass  # noqa: E402
import concourse.mybir as mybir  # noqa: E402
from concourse import bass_utils  # noqa: E402
from concourse.masks import make_identity  # noqa: E402
from concourse.tile import TileContext  # noqa: E402

B, S, DM, H, DH = 2, 1024, 512, 8, 64
PCLIP = 32          # max relative position
NB = S // 128       # q-blocks per sequence
EPS = 1e-6
NEG = -1e30
BF = mybir.dt.bfloat16
F32 = mybir.dt.float32
BF_NP = ml_dtypes.bfloat16
REPLICA_GROUPS = [[0, 1, 2, 3], [4, 5, 6, 7]]


def _sincos_table():
    n = 2 * PCLIP + 1
    pos = np.arange(n)[:, None].astype(np.float32)
    div = np.exp(np.arange(0, DH, 2).astype(np.float32) * (-math.log(10000.0) / DH))
    pe = np.zeros((n, DH), dtype=np.float32)
    pe[:, 0::2] = np.sin(pos * div)
    pe[:, 1::2] = np.cos(pos * div)
    return pe  # [65, 64]


def build(loop=1):
    nc = bacc.Bacc(
        "TRN2",
        target_bir_lowering=False,
        debug=False,
        enable_asserts=False,
        num_devices=8,
    )
    xqT = nc.dram_tensor("xqT", [128, 4, S], BF, kind="ExternalInput")
    kxT = nc.dram_tensor("kxT", [128, 4, S], BF, kind="ExternalInput")
    vxT = nc.dram_tensor("vxT", [128, 4, S], BF, kind="ExternalInput")
    wqT = nc.dram_tensor("wqT", [128, 4, 128], BF, kind="ExternalInput")
    wkT = nc.dram_tensor("wkT", [128, 4, 128], BF, kind="ExternalInput")
    wvT = nc.dram_tensor("wvT", [128, 4, 128], BF, kind="ExternalInput")
    wqb = nc.dram_tensor("wqb", [1, 128], BF, kind="ExternalInput")
    wkb = nc.dram_tensor("wkb", [1, 128], BF, kind="ExternalInput")
    wvb = nc.dram_tensor("wvb", [1, 128], BF, kind="ExternalInput")
    woT = nc.dram_tensor("woT", [128, DM], BF, kind="ExternalInput")
    resid = nc.dram_tensor("resid", [256, DM], F32, kind="ExternalInput")
    lnw = nc.dram_tensor("lnw", [1, DM], F32, kind="ExternalInput")
    lnb = nc.dram_tensor("lnb", [1, DM], F32, kind="ExternalInput")
    tabT = nc.dram_tensor("tabT", [DH, 33], BF, kind="ExternalInput")
    w33 = nc.dram_tensor("w33", [64, DM], BF, kind="ExternalInput")
    bg = nc.dram_tensor("bg", [128, 160], BF, kind="ExternalInput")
    out = nc.dram_tensor("out", [256, DM], F32, kind="ExternalOutput")

    # The heavy odd q-blocks run first so their ReduceScatter overlaps the
    # lighter even-block attention iterations; h is the inner axis.
    ITERS = [(h, i) for i in (1, 3, 5, 7, 0, 2, 4, 6) for h in range(2)]

    def emit_body():
      with TileContext(nc) as tc:
        import contextlib

        with contextlib.ExitStack() as ctx:
            consts = ctx.enter_context(tc.tile_pool(name="consts", bufs=1))
            big = ctx.enter_context(tc.tile_pool(name="big", bufs=1))
            work = ctx.enter_context(tc.tile_pool(name="work", bufs=6))
            pwork = ctx.enter_context(tc.tile_pool(name="pwork", bufs=4))
            psS = ctx.enter_context(tc.tile_pool(name="psS", bufs=2, space="PSUM"))
            psA = ctx.enter_context(tc.tile_pool(name="psA", bufs=2, space="PSUM"))
            dram = ctx.enter_context(tc.tile_pool(name="dram", bufs=1, space="DRAM"))

            # ---------------- constant / input loads ----------------
            # critical path first: zeros/wq/wk/tabT -> xq -> zeroed swin scratch
            def load_w(t, tb):
                w_sb = consts.tile([128, 4, 128], BF, name=t.name + "_sb")
                nc.sync.dma_start(out=w_sb, in_=t.ap())
                w_b = consts.tile([1, 128], BF, name=t.name + "_b")
                nc.sync.dma_start(out=w_b, in_=tb.ap())
                return w_sb, w_b

            zeros_sb = consts.tile([128, 16 * 160], BF)
            nc.vector.memset(zeros_sb, 0.0)
            wq_sb, wq_b = load_w(wqT, wqb)
            wk_sb, wk_b = load_w(wkT, wkb)
            tabT_sb = consts.tile([128, 33], BF)
            nc.sync.dma_start(out=tabT_sb[0:64], in_=tabT.ap())
            nc.sync.dma_start(out=tabT_sb[64:128], in_=tabT.ap())
            xq_sb = big.tile([128, 4, S], BF)
            nc.sync.dma_start(out=xq_sb, in_=xqT.ap())

            # ---------------- DRAM scratch ----------------
            swin_all = dram.tile([128, 16 * 160], BF, tag="swin_all")
            nc.sync.dma_start(out=swin_all, in_=zeros_sb)

            kx_sb = big.tile([128, 4, S], BF)
            nc.scalar.dma_start(out=kx_sb, in_=kxT.ap())
            vx_sb = big.tile([128, 4, S], BF)
            nc.scalar.dma_start(out=vx_sb, in_=vxT.ap())
            wv_sb, wv_b = load_w(wvT, wvb)
            wo_sb = consts.tile([128, DM], BF)
            nc.sync.dma_start(out=wo_sb, in_=woT.ap())
            w33_sb = consts.tile([64, DM], BF)
            nc.sync.dma_start(out=w33_sb, in_=w33.ap())
            lnw_sb = consts.tile([128, DM], F32)
            nc.sync.dma_start(
                out=lnw_sb,
                in_=bass.AP(tensor=lnw.ap().tensor, offset=0, ap=[[0, 128], [1, DM]]),
            )
            lnb_sb = consts.tile([128, DM], F32)
            nc.sync.dma_start(
                out=lnb_sb,
                in_=bass.AP(tensor=lnb.ap().tensor, offset=0, ap=[[0, 128], [1, DM]]),
            )
            ones_sb = consts.tile([1, S], BF)
            nc.vector.memset(ones_sb, 1.0)
            ident = consts.tile([128, 128], BF)
            make_identity(nc, ident)
            bg_sb = consts.tile([128, 160], BF)
            nc.sync.dma_start(out=bg_sb, in_=bg.ap())
            pwinA = dram.tile([128, 8 * 160], BF, tag="pwinA")
            pwinB = dram.tile([128, 8 * 160], BF, tag="pwinB")
            rs_in = dram.tile([S, DM], BF, tag="rs_in")
            rs_out = dram.tile([256, DM], BF, tag="rs_out")

            # ---------------- Q^T / K^T projections ----------------
            qT_sb = big.tile([128, S], BF)
            kT_sb = big.tile([128, S], BF)

            def proj(dst, src, w_sb, w_b):
                for half in range(2):
                    sl = slice(512 * half, 512 * half + 512)
                    ps = psS.tile([128, 512], F32, tag="S1", name="ps_proj")
                    for kc in range(4):
                        nc.tensor.matmul(
                            ps, w_sb[:, kc], src[:, kc, sl],
                            start=(kc == 0), stop=False,
                        )
                    nc.tensor.matmul(ps, w_b, ones_sb[:, 0:512], start=False, stop=True)
                    nc.scalar.copy(out=dst[:, sl], in_=ps)

            proj(qT_sb, xq_sb, wq_sb, wq_b)

            # ---------------- band windows, all up front ----------------
            # scores band: delta[q, jj] = (q . tab[jj] - q . tab[0]) for the 33
            # diagonals; scattered diagonally over a -1e30/0 causal background.
            deltall = big.tile([128, 16, 33], BF)
            qtr_all = psS.tile([128, 2, 512], F32, tag="S2", name="qtr_all")
            for h in range(2):
                hsl = slice(64 * h, 64 * h + 64)
                for i in range(NB):
                    nc.tensor.matmul(
                        qtr_all[:, h, 33 * i : 33 * i + 33],
                        qT_sb[hsl, 128 * i : 128 * i + 128],
                        tabT_sb[hsl],
                        start=True, stop=True,
                    )
            for h in range(2):
                for i in range(NB):
                    it = ITERS.index((h, i))
                    q0 = work.tile([128, 1], F32, tag="q0", name="q0")
                    nc.vector.tensor_copy(out=q0, in_=qtr_all[:, h, 33 * i : 33 * i + 1])
                    nc.vector.tensor_scalar(
                        out=deltall[:, it],
                        in0=qtr_all[:, h, 33 * i : 33 * i + 33],
                        scalar1=q0, scalar2=None,
                        op0=mybir.AluOpType.subtract,
                    )
            nc.sync.dma_start(
                out=bass.AP(
                    tensor=swin_all.tensor, offset=swin_all.offset,
                    ap=[[2561, 128], [160, 16], [1, 33]],
                ),
                in_=deltall,
            )
            wtall = big.tile([128, 16, 160], BF)
            nc.sync.dma_start(out=wtall, in_=swin_all)
            for it in range(16):
                nc.vector.tensor_add(wtall[:, it], wtall[:, it], bg_sb)

            proj(kT_sb, kx_sb, wk_sb, wk_b)

            # ---------------- V projection (natural layout) ----------------
            v_sb = big.tile([128, NB, 128], BF)
            for rb in range(NB):
                psv = psS.tile([128, 128], F32, tag="S1", name="ps_v")
                for kc in range(4):
                    nc.tensor.matmul(
                        psv, vx_sb[:, kc, 128 * rb : 128 * rb + 128], wv_sb[:, kc],
                        start=(kc == 0), stop=False,
                    )
                nc.tensor.matmul(psv, ones_sb[:, 0:128], wv_b, start=False, stop=True)
                nc.gpsimd.tensor_copy(out=v_sb[:, rb], in_=psv)

            # ---------------- attention main loop + deferred rel-V ---------
            attnT = big.tile([128, S], BF)
            pwalls = {0: big.tile([128, 8, 160], BF, name="pwallA"),
                      1: big.tile([128, 8, 160], BF, name="pwallB")}
            nc.vector.memset(pwalls[1][:, 0:2], 0.0)  # i==0 zero left margins

            def main_iter(itl, it, h, i, pwall, pe_transpose=False):
                hsl = slice(64 * h, 64 * h + 64)
                Lk = 128 * (i + 1)
                q_lhsT = qT_sb[hsl, 128 * i : 128 * i + 128]
                Sp = psS.tile(
                    [128, 512 if i < 4 else S], F32,
                    tag="S1" if i < 4 else "S2", name="Sp",
                )
                for c0 in range(0, Lk, 512):
                    w = min(512, Lk - c0)
                    nc.tensor.matmul(
                        Sp[:, c0 : c0 + w], q_lhsT, kT_sb[hsl, c0 : c0 + w],
                        start=True, stop=True,
                    )
                if i == 0:
                    nc.vector.tensor_add(
                        Sp[:, 0:128], Sp[:, 0:128], wtall[:, it, 32:160]
                    )
                else:
                    c0 = 128 * i - 32
                    nc.vector.tensor_add(
                        Sp[:, c0 : c0 + 160], Sp[:, c0 : c0 + 160], wtall[:, it]
                    )
                P = pwork.tile([128, S], BF, tag="P", name="P")
                Zs = work.tile([128, 1], F32, tag="Z", name="Zs")
                nc.scalar.activation(
                    out=P[:, 0:Lk], in_=Sp[:, 0:Lk],
                    func=mybir.ActivationFunctionType.Exp, accum_out=Zs,
                )
                rz = work.tile([128, 1], F32, tag="rz", name="rz")
                nc.vector.reciprocal(out=rz, in_=Zs)
                nc.vector.tensor_scalar_mul(P[:, 0:Lk], P[:, 0:Lk], rz)
                if i == 0:
                    nc.vector.tensor_copy(out=pwall[:, itl, 32:160], in_=P[:, 0:128])
                else:
                    nc.vector.tensor_copy(
                        out=pwall[:, itl], in_=P[:, 128 * i - 32 : 128 * i + 128]
                    )
                ovT = psA.tile([128, 128], F32, tag="ps512", name="ovT")
                for kb in range(i + 1):
                    ptT = work.tile([128, 128], BF, tag="ptT", name="ptT")
                    if pe_transpose:
                        ptT_ps = psS.tile(
                            [128, 128], BF,
                            tag="S2" if i < 4 else "S1", name="ptT_ps",
                        )
                        nc.tensor.transpose(
                            ptT_ps, P[:, 128 * kb : 128 * kb + 128], ident
                        )
                        nc.scalar.copy(out=ptT, in_=ptT_ps)
                    else:
                        nc.sync.dma_start_transpose(
                            ptT, P[:, 128 * kb : 128 * kb + 128]
                        )
                    nc.tensor.matmul(
                        ovT[hsl], v_sb[:, kb, hsl], ptT,
                        start=(kb == 0), stop=(kb == i),
                    )
                nc.vector.tensor_copy(
                    out=attnT[hsl, 128 * i : 128 * i + 128], in_=ovT[hsl]
                )

            def rel_tail(group_iters, pwall, pwin):
                # single batched skew-gather of all 8 windows' probability
                # bands; out_rel = T33[0] + Pband @ (T33[1:] - T33[0]) (row
                # sums are 1), with both terms folded downstream: the constant
                # into `resid` on the host, the band product into the partial
                # output projection via the host-precomputed w33.
                nc.sync.dma_start(out=pwin, in_=pwall)
                pcall = work.tile([128, 8, 32], BF, tag="pcall", name="pcall")
                nc.sync.dma_start(
                    out=pcall,
                    in_=bass.AP(
                        tensor=pwin.tensor, offset=pwin.offset + 1,
                        ap=[[1281, 128], [160, 8], [1, 32]],
                    ),
                )
                pcTs_of = {}
                for bt in range(4):  # transpose both heads' bands of block i
                    pcT_ps = psS.tile([64, 128], BF, tag="S1", name="pcT_ps")
                    nc.tensor.transpose(pcT_ps, pcall[:, 2 * bt : 2 * bt + 2], ident)
                    pcTs = work.tile([64, 128], BF, tag="pcT", name="pcTs")
                    nc.scalar.copy(out=pcTs, in_=pcT_ps)
                    pcTs_of[group_iters[2 * bt][1]] = pcTs
                return pcTs_of

            def partial(rb, pcTs):
                pp = psA.tile([128, DM], F32, tag="ps512", name="pp")
                nc.tensor.matmul(
                    pp, attnT[:, 128 * rb : 128 * rb + 128], wo_sb,
                    start=True, stop=False,
                )
                nc.tensor.matmul(
                    pp, pcTs[0:32], w33_sb[0:32], start=False, stop=False
                )
                nc.tensor.matmul(
                    pp, pcTs[32:64], w33_sb[32:64], start=False, stop=True
                )
                pps = work.tile([128, DM], BF, tag="pps", name="pps")
                nc.scalar.copy(out=pps, in_=pp)
                nc.sync.dma_start(out=rs_in[128 * rb : 128 * rb + 128, :], in_=pps)

            for itl, (h, i) in enumerate(ITERS[0:8]):
                main_iter(itl, itl, h, i, pwalls[0])
            pcTs_odd = rel_tail(ITERS[0:8], pwalls[0], pwinA)
            for rb in (1, 3, 5, 7):
                partial(rb, pcTs_odd[rb])
            for itl, (h, i) in enumerate(ITERS[8:16]):
                main_iter(itl, itl + 8, h, i, pwalls[1])
            pcTs_even = rel_tail(ITERS[8:16], pwalls[1], pwinB)
            for rb in (0, 2, 4, 6):
                partial(rb, pcTs_even[rb])
            nc.gpsimd.collective_compute(
                "ReduceScatter", mybir.AluOpType.add,
                ins=[rs_in.opt()], outs=[rs_out.opt()],
                replica_groups=REPLICA_GROUPS,
            )

            # ---------------- residual + LayerNorm on this core's quarter ---
            for rb in (0, 1):
                rsl = slice(128 * rb, 128 * rb + 128)
                rsb = work.tile([128, DM], BF, tag="rsb", name="rsb")
                nc.sync.dma_start(out=rsb, in_=rs_out[rsl])
                rst = work.tile([128, DM], F32, tag="rst", name="rst")
                nc.sync.dma_start(out=rst, in_=resid.ap()[rsl])
                xsb = work.tile([128, DM], F32, tag="xsb", name="xsb")
                nc.vector.tensor_add(xsb, rst, rsb)
                stats = work.tile([128, 6], F32, tag="stats", name="stats")
                nc.vector.bn_stats(out=stats, in_=xsb)
                mv = work.tile([128, 2], F32, tag="mv", name="mv")
                nc.vector.bn_aggr(out=mv, in_=stats)
                stde = work.tile([128, 1], F32, tag="stde", name="stde")
                nc.scalar.activation(
                    out=stde, in_=mv[:, 1:2],
                    func=mybir.ActivationFunctionType.Sqrt,
                    scale=float(DM) / float(DM - 1),
                )
                nc.vector.tensor_scalar_add(stde, stde, EPS)
                rstd = work.tile([128, 1], F32, tag="rstd", name="rstd")
                nc.vector.reciprocal(out=rstd, in_=stde)
                ysb = work.tile([128, DM], F32, tag="ysb", name="ysb")
                nc.vector.tensor_scalar(
                    out=ysb, in0=xsb, scalar1=mv[:, 0:1], scalar2=rstd,
                    op0=mybir.AluOpType.subtract, op1=mybir.AluOpType.mult,
                )
                nc.vector.tensor_mul(ysb, ysb, lnw_sb)
                nc.vector.tensor_add(ysb, ysb, lnb_sb)
                nc.sync.dma_start(out=out.ap()[rsl], in_=ysb)

    for _ in range(loop):
        emit_body()
    nc.compile()
    return nc


def make_in_maps(query, key, value, Wq, bq, Wk, bk, Wv, bv, Wo, bo, ln_w, ln_b):
    table = _sincos_table()
    tabT = np.ascontiguousarray(table.T[:, :33]).astype(BF_NP)  # [64, 33]
    t33d = table[1:33] - table[0:1]                              # [32, 64]
    rel_const = np.tile(table[0], H) @ Wo.T                      # [512]
    bgp = np.zeros((128, 160), dtype=np.float32)
    cidx = np.arange(160)[None, :]
    pidx = np.arange(128)[:, None]
    bgp[cidx > pidx + 32] = NEG
    bgp = bgp.astype(BF_NP)

    def pmajor(a):  # [512, n] -> [128, 4, n] partition-major
        n = a.shape[1]
        return np.ascontiguousarray(
            a.reshape(4, 128, n).transpose(1, 0, 2)
        ).astype(BF_NP)

    in_maps = []
    for c in range(8):
        b, g = divmod(c, 4)
        sl = slice(128 * g, 128 * g + 128)
        resid = (
            query[b, 256 * g : 256 * g + 256, :] + bo[None, :] + rel_const[None, :]
        ).astype(np.float32)
        woT_sl = Wo.T[sl, :]
        w33_c = np.concatenate([t33d @ woT_sl[0:64], t33d @ woT_sl[64:128]], axis=0)
        in_maps.append(
            {
                "xqT": pmajor(np.ascontiguousarray(query[b].T)),
                "kxT": pmajor(np.ascontiguousarray(key[b].T)),
                "vxT": pmajor(np.ascontiguousarray(value[b].T)),
                "wqT": pmajor(np.ascontiguousarray(Wq.T[:, sl] / 8.0)),
                "wkT": pmajor(np.ascontiguousarray(Wk.T[:, sl])),
                "wvT": pmajor(np.ascontiguousarray(Wv.T[:, sl])),
                "wqb": (bq[sl] / 8.0)[None, :].astype(BF_NP),
                "wkb": bk[sl][None, :].astype(BF_NP),
                "wvb": bv[sl][None, :].astype(BF_NP),
                "woT": np.ascontiguousarray(Wo.T[sl, :]).astype(BF_NP),
                "resid": resid,
                "lnw": np.asarray(ln_w, np.float32).reshape(1, DM),
                "lnb": np.asarray(ln_b, np.float32).reshape(1, DM),
                "tabT": tabT,
                "w33": np.ascontiguousarray(w33_c).astype(BF_NP),
                "bg": bgp,
            }
        )
    return in_maps


def assemble(results):
    full = np.zeros((B, S, DM), dtype=np.float32)
    for c in range(8):
        b, g = divmod(c, 4)
        full[b, 256 * g : 256 * g + 256, :] = results[c]["out"]
    return full


_CACHE = {}


def kernel(**inputs):
    inputs = {k: np.asarray(v) for k, v in inputs.items()}
    if "nc" not in _CACHE:
        _CACHE["nc"] = build()
    nc = _CACHE["nc"]
    in_maps = make_in_maps(**inputs)
    res = bass_utils.run_bass_kernel_spmd(nc, in_maps, core_ids=list(range(8)))
    return assemble(res.results)


if __name__ == "__main__":
    rng = np.random.default_rng(0)
    dummy = {
        "query": rng.normal(size=(B, S, DM)).astype(np.float32),
        "key": rng.normal(size=(B, S, DM)).astype(np.float32),
        "value": rng.normal(size=(B, S, DM)).astype(np.float32),
        "Wq": rng.normal(size=(DM, DM)).astype(np.float32) * 0.02,
        "bq": np.zeros(DM, np.float32),
        "Wk": rng.normal(size=(DM, DM)).astype(np.float32) * 0.02,
        "bk": np.zeros(DM, np.float32),
        "Wv": rng.normal(size=(DM, DM)).astype(np.float32) * 0.02,
        "bv": np.zeros(DM, np.float32),
        "Wo": rng.normal(size=(DM, DM)).astype(np.float32) * 0.02,
        "bo": np.zeros(DM, np.float32),
        "ln_w": np.ones(DM, np.float32),
        "ln_b": np.zeros(DM, np.float32),
    }
    out = kernel(**dummy)
    print("kernel ran, out shape", out.shape, "mean", float(np.abs(out).mean()))
